# revision 1
# baseline (speedup 1.0000x reference)
"""Kinematics LSTM decoder on 8 trn2 NeuronCores.

Strategy: model-parallel over the 4608 gate dim (576 gate cols / core,
= 144 h cols / core), all LSTM weights SBUF-resident (18.7MB/core).
Recurrence: 25 steps x 6 cells; per-cell AllGather of the (transposed)
h slice through DRAM bounce buffers. Gates layout [batch, gatecols]
with per-core gate column order [i f o g]; matmuls run as float32r
(full PE rate, fp32 storage). Layers 2,3 share weights AND input -> batch
stacked (M=128); shared-weight layers 4,5 are sequential so unstacked.

Self-contained: hardcodes shapes; host-side numpy only reorders/slices
weights and shards inputs.
"""
import numpy as np

B, T_ENC, D_IN, H, T_OUT = 64, 49, 54, 1152, 25
NC_ = 8          # cores
HS = H // NC_    # 144 h cols per core
GS = 4 * HS      # 576 gate cols per core
NK = H // 128    # 9 contraction chunks
F32 = None       # set after imports

_compiled = None


def _build():
    import concourse.bass as bass
    import concourse.bacc as bacc
    import concourse.tile as tile
    import concourse.mybir as mybir

    f32 = mybir.dt.float32
    f32r = mybir.dt.float32r
    AF = mybir.ActivationFunctionType
    OP = mybir.AluOpType

    nc = bacc.Bacc("TRN2", target_bir_lowering=False, debug=False,
                   num_devices=NC_)

    # ---- DRAM I/O ----
    din = {}
    def dram_in(name, shape):
        din[name] = nc.dram_tensor(name, list(shape), f32, kind="ExternalInput")
        return din[name]

    def dram_in_r(name, shape):
        din[name] = nc.dram_tensor(name, list(shape), f32r, kind="ExternalInput")
        return din[name]

    dram_in_r("w0x", (54, GS))
    for tag in ("0h", "1x", "1h", "Ax", "Ah", "Lx", "Lh"):
        dram_in_r("w" + tag, (NK, 128, GS))
    for tag in "01AL":
        dram_in("b" + tag, (1, GS))
    for hn, hw in (("leg1", 12), ("leg2", 12), ("spine", 12),
                   ("arm1", 9), ("arm2", 9)):
        dram_in("wh_" + hn, (NK, 128, hw))
    dram_in("hbias", (B, D_IN))          # head biases pre-broadcast
    dram_in("hs_sl", (B, T_ENC, HS))
    dram_in("cs_sl", (B, T_ENC, HS))
    dram_in("gts_sl", (B, HS))
    dram_in("p0", (B, D_IN))
    dram_in("ident", (128, 128))
    dram_in_r("zeros", (128, NK, 2 * B))
    out_d = nc.dram_tensor("out", [B, T_OUT, D_IN], f32, kind="ExternalOutput")

    RG = [list(range(NC_))]

    with tile.TileContext(nc) as tc:
        with tc.tile_pool(name="wpool", bufs=1) as wp, \
             tc.tile_pool(name="state", bufs=1) as st, \
             tc.tile_pool(name="work", bufs=3) as wk, \
             tc.tile_pool(name="hnewp", bufs=2) as hp, \
             tc.tile_pool(name="psg", bufs=2, space="PSUM") as psg, \
             tc.tile_pool(name="pst", bufs=2, space="PSUM") as pst, \
             tc.tile_pool(name="psh", bufs=1, space="PSUM") as psh, \
             tc.tile_pool(name="dram", bufs=6, space="DRAM") as dp:

            # ---- load weights ----
            w_sb = {}
            w_sb["0x"] = wp.tile([54, GS], f32r, tag="w0x", name="w0x")
            nc.sync.dma_start(w_sb["0x"][:], din["w0x"][:])
            for tag in ("0h", "1x", "1h", "Ax", "Ah", "Lx", "Lh"):
                w_sb[tag] = wp.tile([128, NK, GS], f32r, tag="w" + tag, name="w" + tag)
                nc.sync.dma_start(
                    w_sb[tag][:], din["w" + tag][:].rearrange("c k n -> k c n"))
            b_sb = {}
            for tag in "01AL":
                b_sb[tag] = wp.tile([1, GS], f32, tag="b" + tag, name="b" + tag)
                nc.sync.dma_start(b_sb[tag][:], din["b" + tag][:])
            wh_sb = {}
            for hn, hw in (("leg1", 12), ("leg2", 12), ("spine", 12),
                           ("arm1", 9), ("arm2", 9)):
                wh_sb[hn] = wp.tile([128, NK, hw], f32, tag="wh" + hn, name="wh" + hn)
                nc.sync.dma_start(
                    wh_sb[hn][:], din["wh_" + hn][:].rearrange("c k n -> k c n"))
            hbias = wp.tile([B, D_IN], f32, tag="hbias", name="hbias")
            nc.sync.dma_start(hbias[:], din["hbias"][:])
            ident = wp.tile([128, 128], f32, tag="ident", name="ident")
            nc.sync.dma_start(ident[:], din["ident"][:])
            ones = wp.tile([1, 128], f32, tag="ones", name="ones")
            nc.vector.memset(ones[:], 1.0)

            # ---- persistent state ----
            hT0 = st.tile([128, NK, B], f32r, tag="hT0", name="hT0")        # h0.T
            hT1 = st.tile([128, NK, 2 * B], f32r, tag="hT1", name="hT1")    # h1.T duplicated
            hTA = st.tile([128, NK, 2 * B], f32r, tag="hTA", name="hTA")    # h2.T | h3.T
            hTL = st.tile([128, NK, 2 * B], f32r, tag="hTL", name="hTL")    # h4.T | h5.T
            c_st = {0: st.tile([B, HS], f32, tag="c0", name="c0"),
                    1: st.tile([B, HS], f32, tag="c1", name="c1"),
                    "A": st.tile([2 * B, HS], f32, tag="cA", name="cA"),
                    4: st.tile([B, HS], f32, tag="c4", name="c4"),
                    5: st.tile([B, HS], f32, tag="c5", name="c5")}
            x0b = st.tile([B, D_IN], f32, tag="x0b", name="x0b")
            x0T = st.tile([D_IN, B], f32r, tag="x0T", name="x0T")

            nc.sync.dma_start(hTA[:], din["zeros"][:])
            nc.sync.dma_start(hTL[:], din["zeros"][:])
            nc.vector.memset(c_st["A"][:], 0.0)
            nc.vector.memset(c_st[4][:], 0.0)
            nc.vector.memset(c_st[5][:], 0.0)

            r32 = lambda ap: ap.bitcast(f32r)

            def transpose_to(dst_dram_slices, src_sb, rows, cols):
                """src_sb [rows<=128, cols] -> dram bounce rows=cols x rows,
                dst_dram_slices: list of (dram_ap, col_lo, col_hi) col splits
                of the transposed [cols, rows]."""
                done = 0
                while done < cols:
                    n = min(128, cols - done)
                    pt = pst.tile([128, 128], f32, tag="pt", name="pt")
                    nc.tensor.transpose(pt[0:n, 0:rows],
                                        src_sb[0:rows, done:done + n],
                                        ident[0:rows, 0:rows])
                    cp = wk.tile([128, 128], f32r, tag="tcp", name="tcp")
                    nc.scalar.copy(cp[0:n, 0:rows], pt[0:n, 0:rows])
                    for (dap, lo, hi) in dst_dram_slices:
                        nc.sync.dma_start(dap[done:done + n, :],
                                          cp[0:n, lo:hi])
                    done += n

            def allgather(n_rows):
                gin = dp.tile([n_rows, B], f32r, tag="agin", name="agin")
                gout = dp.tile([NC_ * n_rows, B], f32r, tag="agout", name="agout")
                return gin, gout

            def do_ag(gin, gout):
                nc.gpsimd.collective_compute(
                    "AllGather", OP.bypass, replica_groups=RG,
                    ins=[gin[:].opt()], outs=[gout[:].opt()])

            # ---- prologue: means ----
            accs = {}
            for nm in ("hs_sl", "cs_sl"):
                acc = wk.tile([B, HS], f32, tag="acc", name="acc" + nm)
                nc.vector.memset(acc[:], 0.0)
                for t in range(T_ENC):
                    pl = wk.tile([B, HS], f32, tag="plane", name="plane")
                    nc.sync.dma_start(pl[:], din[nm][:, t, :])
                    nc.vector.tensor_tensor(acc[:], acc[:], pl[:], op=OP.add)
                accs[nm] = acc
            # c_init
            nc.scalar.mul(c_st[0][:], accs["cs_sl"][:], 1.0 / T_ENC)
            nc.vector.tensor_copy(c_st[1][:], c_st[0][:])
            # h0, h1
            h0m = wk.tile([B, HS], f32, tag="h0m", name="h0m")
            nc.scalar.mul(h0m[:], accs["hs_sl"][:], 1.0 / T_ENC)
            gts = wk.tile([B, HS], f32, tag="gts", name="gts")
            nc.sync.dma_start(gts[:], din["gts_sl"][:])
            h1m = wk.tile([B, HS], f32, tag="h1m", name="h1m")
            nc.vector.tensor_tensor(h1m[:], accs["hs_sl"][:], gts[:], op=OP.add)
            nc.scalar.mul(h1m[:], h1m[:], 1.0 / (T_ENC + 1))

            for (src, dsts) in ((h0m, [(hT0, 0, B)]),
                                (h1m, [(hT1, 0, B), (hT1, B, 2 * B)])):
                gin, gout = allgather(HS)
                transpose_to([(gin[:], 0, B)], src, B, HS)
                do_ag(gin, gout)
                for (dst, lo, hi) in dsts:
                    nc.sync.dma_start(
                        dst[:, :, lo:hi],
                        gout[:].rearrange("(c k) n -> k c n", k=128))

            # x0
            nc.sync.dma_start(x0b[:], din["p0"][:])
            ptp = pst.tile([128, 128], f32, tag="pt", name="pt")
            nc.tensor.transpose(ptp[0:D_IN, 0:B], x0b[0:B, 0:D_IN],
                                ident[0:B, 0:B])
            nc.scalar.copy(x0T[:], ptp[0:D_IN, 0:B])

            # ---- helpers for the recurrence ----
            def gate_mms(g0, g1, rows, wtag, x_chunks, h_chunks):
                """accumulate x@WxT + h@WhT + bias into g0 (cols 0:288) and
                g1 (288:576). x_chunks/h_chunks: list of (lhsT_ap, rhs_tile_key)
                pairs... actually (lhsT_ap, wkey, chunk_idx)."""
                first = True
                items = h_chunks + x_chunks
                n = len(items)
                for idx, (lhsT, wkey, c) in enumerate(items):
                    if wkey == "0x":
                        r0 = w_sb["0x"][0:54, 0:288]
                        r1 = w_sb["0x"][0:54, 288:GS]
                    else:
                        r0 = w_sb[wkey][:, c, 0:288]
                        r1 = w_sb[wkey][:, c, 288:GS]
                    nc.tensor.matmul(g0[0:rows, :], r32(lhsT), r32(r0),
                                     start=first, stop=False)
                    nc.tensor.matmul(g1[0:rows, :], r32(lhsT), r32(r1),
                                     start=first, stop=False)
                    first = False
                # bias
                nc.tensor.matmul(g0[0:rows, :], ones[0:1, 0:rows],
                                 b_sb[wtag][0:1, 0:288],
                                 start=False, stop=True)
                nc.tensor.matmul(g1[0:rows, :], ones[0:1, 0:rows],
                                 b_sb[wtag][0:1, 288:GS],
                                 start=False, stop=True)

            def elementwise(g0, g1, rows, c_tile, crange):
                """gates [i f | o g]; returns h_new sbuf tile [rows, HS]"""
                sif = wk.tile([128, 2 * HS], f32, tag="sif", name="sif")
                nc.scalar.activation(sif[0:rows, :], g0[0:rows, :], AF.Sigmoid)
                so = wk.tile([128, HS], f32, tag="so", name="so")
                nc.scalar.activation(so[0:rows, :], g1[0:rows, 0:HS], AF.Sigmoid)
                tg = wk.tile([128, HS], f32, tag="tg", name="tg")
                nc.scalar.activation(tg[0:rows, :], g1[0:rows, HS:2 * HS], AF.Tanh)
                t1 = wk.tile([128, HS], f32, tag="t1", name="t1")
                cs = c_tile[crange[0]:crange[1], :]
                nc.vector.tensor_tensor(t1[0:rows, :], sif[0:rows, HS:2 * HS],
                                        cs, op=OP.mult)
                t2 = wk.tile([128, HS], f32, tag="t2", name="t2")
                nc.vector.tensor_tensor(t2[0:rows, :], sif[0:rows, 0:HS],
                                        tg[0:rows, :], op=OP.mult)
                nc.vector.tensor_tensor(cs, t1[0:rows, :], t2[0:rows, :],
                                        op=OP.add)
                tc_ = wk.tile([128, HS], f32, tag="tc", name="tc")
                nc.scalar.activation(tc_[0:rows, :], cs, AF.Tanh)
                hn = hp.tile([128, HS], f32, tag="hnew", name="hnew")
                nc.vector.tensor_tensor(hn[0:rows, :], so[0:rows, :],
                                        tc_[0:rows, :], op=OP.mult)
                return hn

            def dma_back(gout, dst, lo, hi):
                nc.sync.dma_start(
                    dst[:, :, lo:hi],
                    gout[:].rearrange("(c k) n -> k c n", k=128))

            # ---- recurrence ----
            for t in range(T_OUT):
                # L0
                g0 = psg.tile([128, 288], f32, tag="g0", name="g0")
                g1 = psg.tile([128, 288], f32, tag="g1", name="g1")
                gate_mms(g0, g1, B, "0",
                         x_chunks=[(x0T[0:54, 0:B], "0x", 0)],
                         h_chunks=[(hT0[:, c, :], "0h", c) for c in range(NK)])
                hn0 = elementwise(g0, g1, B, c_st[0], (0, B))
                gin0, gout0 = allgather(HS)
                transpose_to([(gin0[:], 0, B)], hn0, B, HS)
                do_ag(gin0, gout0)
                dma_back(gout0, hT0, 0, B)

                # L1 (x = new h0)
                g0 = psg.tile([128, 288], f32, tag="g0", name="g0")
                g1 = psg.tile([128, 288], f32, tag="g1", name="g1")
                gate_mms(g0, g1, B, "1",
                         x_chunks=[(hT0[:, c, :], "1x", c) for c in range(NK)],
                         h_chunks=[(hT1[:, c, 0:B], "1h", c) for c in range(NK)])
                hn1 = elementwise(g0, g1, B, c_st[1], (0, B))
                gin1, gout1 = allgather(HS)
                transpose_to([(gin1[:], 0, B)], hn1, B, HS)
                do_ag(gin1, gout1)
                dma_back(gout1, hT1, 0, B)
                dma_back(gout1, hT1, B, 2 * B)

                # A-pair: layers 2,3 stacked (x = new h1 for BOTH)
                g0 = psg.tile([128, 288], f32, tag="g0", name="g0")
                g1 = psg.tile([128, 288], f32, tag="g1", name="g1")
                gate_mms(g0, g1, 128, "A",
                         x_chunks=[(hT1[:, c, :], "Ax", c) for c in range(NK)],
                         h_chunks=[(hTA[:, c, :], "Ah", c) for c in range(NK)])
                hnA = elementwise(g0, g1, 128, c_st["A"], (0, 128))
                gin2, gout2 = allgather(HS)
                gin3, gout3 = allgather(HS)
                transpose_to([(gin2[:], 0, B), (gin3[:], B, 2 * B)],
                             hnA, 128, HS)
                do_ag(gin2, gout2)
                do_ag(gin3, gout3)
                dma_back(gout2, hTA, 0, B)
                dma_back(gout3, hTA, B, 2 * B)

                # L4 (x = new h3)
                g0 = psg.tile([128, 288], f32, tag="g0", name="g0")
                g1 = psg.tile([128, 288], f32, tag="g1", name="g1")
                gate_mms(g0, g1, B, "L",
                         x_chunks=[(hTA[:, c, B:2 * B], "Lx", c) for c in range(NK)],
                         h_chunks=[(hTL[:, c, 0:B], "Lh", c) for c in range(NK)])
                hn4 = elementwise(g0, g1, B, c_st[4], (0, B))
                gin4, gout4 = allgather(HS)
                transpose_to([(gin4[:], 0, B)], hn4, B, HS)
                do_ag(gin4, gout4)
                dma_back(gout4, hTL, 0, B)

                # L5 (x = new h4)
                g0 = psg.tile([128, 288], f32, tag="g0", name="g0")
                g1 = psg.tile([128, 288], f32, tag="g1", name="g1")
                gate_mms(g0, g1, B, "L",
                         x_chunks=[(hTL[:, c, 0:B], "Lx", c) for c in range(NK)],
                         h_chunks=[(hTL[:, c, B:2 * B], "Lh", c) for c in range(NK)])
                hn5 = elementwise(g0, g1, B, c_st[5], (0, B))
                gin5, gout5 = allgather(HS)
                transpose_to([(gin5[:], 0, B)], hn5, B, HS)
                do_ag(gin5, gout5)
                dma_back(gout5, hTL, B, 2 * B)

                # heads (replicated on every core)
                ph = psh.tile([B, D_IN], f32, tag="ph", name="ph")
                heads = [("leg1", hTA, 0, B, 0, 12),
                         ("leg2", hTA, B, 2 * B, 12, 24),
                         ("spine", hT1, 0, B, 24, 36),
                         ("arm1", hTL, 0, B, 36, 45),
                         ("arm2", hTL, B, 2 * B, 45, 54)]
                for hn_, src, lo, hi, olo, ohi in heads:
                    for c in range(NK):
                        nc.tensor.matmul(ph[:, olo:ohi],
                                         src[:, c, lo:hi].bitcast(f32),
                                         wh_sb[hn_][:, c, :],
                                         start=(c == 0), stop=(c == NK - 1))
                pre = wk.tile([B, D_IN], f32, tag="pre", name="pre")
                nc.vector.tensor_tensor(pre[:], ph[:], hbias[:], op=OP.add)
                nc.vector.tensor_tensor(pre[:], pre[:], x0b[:], op=OP.add)
                nc.sync.dma_start(out_d[:, t, :], pre[:])
                if t < T_OUT - 1:
                    nc.vector.tensor_copy(x0b[:], pre[:])
                    ptq = pst.tile([128, 128], f32, tag="pt", name="pt")
                    nc.tensor.transpose(ptq[0:D_IN, 0:B], pre[0:B, 0:D_IN],
                                        ident[0:B, 0:B])
                    nc.scalar.copy(x0T[:], ptq[0:D_IN, 0:B])

    nc.compile()
    return nc


def _prep_inputs(inputs):
    """slice/reorder per core -> in_maps"""
    gate_off = {"i": 0, "f": H, "g": 2 * H, "o": 3 * H}
    in_maps = []
    hbias = np.concatenate([inputs["b_leg1"], inputs["b_leg2"],
                            inputs["b_spine"], inputs["b_arm1"],
                            inputs["b_arm2"]]).astype(np.float32)
    hbias_b = np.broadcast_to(hbias, (B, D_IN)).copy()
    ident = np.eye(128, dtype=np.float32)
    for j in range(NC_):
        sl = slice(j * HS, (j + 1) * HS)
        sel = np.concatenate([np.arange(gate_off[g] + j * HS,
                                        gate_off[g] + (j + 1) * HS)
                              for g in "ifog"])
        m = {}
        m["w0x"] = np.ascontiguousarray(inputs["Wih0"].T[:, sel])
        for tag, W in (("0h", "Whh0"), ("1x", "Wih1"), ("1h", "Whh1"),
                       ("Ax", "WihA"), ("Ah", "WhhA"),
                       ("Lx", "WihL"), ("Lh", "WhhL")):
            m["w" + tag] = np.ascontiguousarray(
                inputs[W].T[:, sel].reshape(NK, 128, GS))
        for tag, bi, bh in (("0", "bih0", "bhh0"), ("1", "bih1", "bhh1"),
                            ("A", "bihA", "bhhA"), ("L", "bihL", "bhhL")):
            m["b" + tag] = (inputs[bi] + inputs[bh])[sel][None, :].astype(np.float32)
        for hn, wn in (("leg1", "W_leg1"), ("leg2", "W_leg2"),
                       ("spine", "W_spine"), ("arm1", "W_arm1"),
                       ("arm2", "W_arm2")):
            w = inputs[wn]
            m["wh_" + hn] = np.ascontiguousarray(
                w.reshape(NK, 128, w.shape[1]))
        m["hbias"] = hbias_b
        m["hs_sl"] = np.ascontiguousarray(inputs["hidden_states"][:, :, sl])
        m["cs_sl"] = np.ascontiguousarray(inputs["cell_states"][:, :, sl])
        m["gts_sl"] = np.ascontiguousarray(inputs["global_t_state"][:, sl])
        m["p0"] = np.ascontiguousarray(inputs["p"][:, 0, :])
        m["ident"] = ident
        m["zeros"] = np.zeros((128, NK, 2 * B), np.float32)
        m = {k: np.asarray(v, dtype=np.float32) for k, v in m.items()}
        in_maps.append(m)
    return in_maps


def kernel(**inputs):
    global _compiled
    import concourse.bass_utils as bass_utils
    if _compiled is None:
        _compiled = _build()
    in_maps = _prep_inputs(inputs)
    res = bass_utils.run_bass_kernel_spmd(
        _compiled, in_maps, core_ids=list(range(NC_)))
    return np.asarray(res.results[0]["out"], dtype=np.float32)



# revision 3
# speedup vs baseline: 1.1370x; 1.1370x over previous
"""Kinematics LSTM decoder on 8 trn2 NeuronCores — wire-optimized v2.

The axon tunnel moves host->device bytes at ~40MB/s, so the per-call
wall is dominated by input transfer. v2 ships the LSTM weights as int8
(dequantized on device into SBUF-resident f32 tiles; quant sim l2rel
2.5e-3 vs 2e-2 budget) and computes the encoder means host-side, cutting
the payload from ~186MB to ~41MB.

Device strategy (unchanged from v1): model-parallel over the 4608 gate
dim (576 gate cols / core = 144 h cols / core). Recurrence: 25 steps x 6
cells; per-cell AllGather of the transposed h slice through DRAM bounce
buffers. Gates layout [batch, gatecols], per-core col order [i f | o g];
matmuls in f32r. Layers 2,3 share weights AND input -> batch-stacked.
"""
import numpy as np

B, T_ENC, D_IN, H, T_OUT = 64, 49, 54, 1152, 25
NC_ = 8          # cores
HS = H // NC_    # 144 h cols per core
GS = 4 * HS      # 576 gate cols per core
NK = H // 128    # 9 contraction chunks

PERM = [0, 1, 3, 2]  # pytorch gate order (i,f,g,o) -> per-core col order (i,f,o,g)
TAGS = [("0h", "Whh0"), ("1x", "Wih1"), ("1h", "Whh1"),
        ("Ax", "WihA"), ("Ah", "WhhA"), ("Lx", "WihL"), ("Lh", "WhhL")]
TAGIDX = {"0": 0, "1": 1, "A": 2, "L": 3}

_NF = NK * GS
_NH = _NF // 2
_NL = _NF // 4


def _blob_layout():
    layout = [
        ("wscale", 128 * 24 * 4),
        ("bias", 4 * GS * 4),
        ("hb1", D_IN * 4),
        ("h0T", HS * B * 4),
        ("h1T", HS * B * 4),
        ("cin", B * HS * 4),
        ("p0", B * D_IN * 4),
        ("wh8", NK * 128 * 54),
        ("w0x8", 54 * GS),
        ("wq6h", 7 * 128 * _NH),
        ("wq6l", 7 * 128 * _NL),
    ]
    off, d = 0, {}
    for name, nb in layout:
        d[name] = (off, nb)
        off += nb
    return d, off


BLOB_OFF, BLOB_BYTES = _blob_layout()

_compiled = None


def _enable_jax_cache():
    """Persistent XLA executable cache: without it every
    run_bass_kernel_spmd call re-lowers + re-runs the walrus NEFF
    compile (~0.9s/call)."""
    try:
        import jax
        jax.config.update("jax_compilation_cache_dir", "/tmp/bass_jax_cache")
        jax.config.update("jax_persistent_cache_min_entry_size_bytes", -1)
        jax.config.update("jax_persistent_cache_min_compile_time_secs", 0)
    except Exception:
        pass


_enable_jax_cache()


def _build():
    import concourse.bass as bass
    import concourse.bacc as bacc
    import concourse.tile as tile
    import concourse.mybir as mybir

    f32 = mybir.dt.float32
    f32r = mybir.dt.float32r
    bf16 = mybir.dt.bfloat16
    i8 = mybir.dt.int8
    u8 = mybir.dt.uint8
    AF = mybir.ActivationFunctionType
    OP = mybir.AluOpType

    NF = NK * GS          # 5184 flat weight cols per partition
    NH = NF // 2          # 2592
    NL = NF // 4          # 1296

    nc = bacc.Bacc("TRN2", target_bir_lowering=False, debug=False,
                   num_devices=NC_)

    # single per-core input blob; section offsets must match _prep_inputs
    blob = nc.dram_tensor("blob", [BLOB_BYTES], u8, kind="ExternalInput")

    def bsec(name, dt_):
        off, nbytes = BLOB_OFF[name]
        ap = blob[off:off + nbytes]
        return ap if dt_ == u8 else ap.bitcast(dt_)

    out_d = nc.dram_tensor("out", [B, T_OUT, D_IN], bf16, kind="ExternalOutput")

    RG = [list(range(NC_))]

    with tile.TileContext(nc) as tc:
        with tc.tile_pool(name="wpool", bufs=1) as wp, \
             tc.tile_pool(name="stg", bufs=1) as stg, \
             tc.tile_pool(name="state", bufs=1) as st, \
             tc.tile_pool(name="work", bufs=3) as wk, \
             tc.tile_pool(name="hnewp", bufs=2) as hp, \
             tc.tile_pool(name="psg", bufs=2, space="PSUM") as psg, \
             tc.tile_pool(name="pst", bufs=2, space="PSUM") as pst, \
             tc.tile_pool(name="psh", bufs=1, space="PSUM") as psh, \
             tc.tile_pool(name="dram", bufs=6, space="DRAM") as dp:

            # ---- scales / misc constants ----
            wsc = wp.tile([128, 24], f32, tag="wsc", name="wsc")
            nc.sync.dma_start(
                wsc[:], bsec("wscale", f32).rearrange("(p f) -> p f", p=128))
            ones = wp.tile([1, 128], f32, tag="ones", name="ones")
            nc.vector.memset(ones[:], 1.0)
            ones128 = wp.tile([128, 128], f32, tag="ones128", name="ones128")
            nc.vector.memset(ones128[:], 1.0)
            ident = wp.tile([128, 128], f32, tag="ident", name="ident")
            nc.gpsimd.affine_select(ident[:], ones128[:], pattern=[[-1, 128]],
                                    compare_op=OP.is_equal, fill=0.0,
                                    base=0, channel_multiplier=1)
            hb1 = wp.tile([1, D_IN], f32, tag="hb1", name="hb1")
            nc.sync.dma_start(
                hb1[:], bsec("hb1", f32).rearrange("(p f) -> p f", p=1))
            boff = BLOB_OFF["bias"][0]
            b_sb = {}
            for ti, tg in enumerate("01AL"):
                b_sb[tg] = wp.tile([1, GS], f32, tag="b" + tg, name="b" + tg)
                nc.sync.dma_start(
                    b_sb[tg][:],
                    blob[boff + ti * GS * 4:boff + (ti + 1) * GS * 4]
                    .bitcast(f32).rearrange("(p f) -> p f", p=1))

            # ---- unpack int6 weights into SBUF-resident f32r tiles ----
            # per weight w: v = round(w/s)+32 in [1,63]; h=v>>2 (4b), l=v&3 (2b)
            # w = h*(4s) - 32s + l*s
            w_sb = {}
            hoff = BLOB_OFF["wq6h"][0]
            loff = BLOB_OFF["wq6l"][0]
            for i, (tag, _) in enumerate(TAGS):
                sh = stg.tile([128, NH], u8, tag="sth", name="sth" + tag)
                nc.sync.dma_start(
                    sh[:], blob[hoff + i * 128 * NH:hoff + (i + 1) * 128 * NH]
                    .rearrange("(p f) -> p f", p=128))
                sl = stg.tile([128, NL], u8, tag="stl", name="stl" + tag)
                nc.sync.dma_start(
                    sl[:], blob[loff + i * 128 * NL:loff + (i + 1) * 128 * NL]
                    .rearrange("(p f) -> p f", p=128))
                w_sb[tag] = wp.tile([128, NK, GS], f32r, tag="w" + tag, name="w" + tag)
                wf = w_sb[tag][:].rearrange("p a b -> p (a b)")
                s4 = wsc[:, 3 * i:3 * i + 1]
                sm32 = wsc[:, 3 * i + 1:3 * i + 2]
                s1 = wsc[:, 3 * i + 2:3 * i + 3]
                # hi plane
                th = stg.tile([128, NH], u8, tag="th", name="th" + tag)
                nc.vector.tensor_scalar(th[:], sh[:], 15, None, op0=OP.bitwise_and)
                nc.vector.tensor_scalar(wf[:, 0:NH], th[:], s4, sm32,
                                        op0=OP.mult, op1=OP.add)
                nc.vector.tensor_scalar(th[:], sh[:], 4, None,
                                        op0=OP.logical_shift_right)
                nc.vector.tensor_scalar(wf[:, NH:NF], th[:], s4, sm32,
                                        op0=OP.mult, op1=OP.add)
                # lo plane: 4 quarters x 2 halves (small temps)
                NL2 = NL // 2
                for qd in range(4):
                    for hh in range(2):
                        lsrc = sl[:, hh * NL2:(hh + 1) * NL2]
                        dst = wf[:, qd * NL + hh * NL2:qd * NL + (hh + 1) * NL2]
                        tl = stg.tile([128, NL2], u8, tag="tl", name="tl")
                        if qd == 0:
                            nc.vector.tensor_scalar(tl[:], lsrc, 3, None,
                                                    op0=OP.bitwise_and)
                        elif qd == 3:
                            nc.vector.tensor_scalar(tl[:], lsrc, 6, None,
                                                    op0=OP.logical_shift_right)
                        else:
                            nc.vector.tensor_scalar(tl[:], lsrc, 2 * qd, 3,
                                                    op0=OP.logical_shift_right,
                                                    op1=OP.bitwise_and)
                        tf = stg.tile([128, NL2], f32, tag="tf", name="tf")
                        nc.vector.tensor_scalar(tf[:], tl[:], s1, None,
                                                op0=OP.mult)
                        nc.vector.tensor_tensor(dst, dst, tf[:], op=OP.add)

            sg0 = stg.tile([54, GS], i8, tag="stg0", name="stg0x")
            nc.sync.dma_start(
                sg0[:], bsec("w0x8", i8).rearrange("(p f) -> p f", p=54))
            w_sb["0x"] = wp.tile([54, GS], f32r, tag="w0x", name="w0x")
            nc.vector.tensor_scalar(w_sb["0x"][:], sg0[:], wsc[0:54, 21:22],
                                    None, op0=OP.mult)
            sgh = stg.tile([128, NK, 54], i8, tag="stgh", name="stgh")
            nc.sync.dma_start(
                sgh[:], bsec("wh8", i8).rearrange("(c k n) -> k c n",
                                                  c=NK, k=128, n=54))
            wh_all = wp.tile([128, NK, 54], f32, tag="whall", name="whall")
            nc.vector.tensor_scalar(wh_all[:], sgh[:], wsc[:, 22:23],
                                    None, op0=OP.mult)

            # ---- persistent state ----
            hT0 = st.tile([128, NK, B], f32r, tag="hT0", name="hT0")        # h0.T
            hT1 = st.tile([128, NK, 2 * B], f32r, tag="hT1", name="hT1")    # h1.T duplicated
            hTA = st.tile([128, NK, 2 * B], f32r, tag="hTA", name="hTA")    # h2.T | h3.T
            hTL = st.tile([128, NK, 2 * B], f32r, tag="hTL", name="hTL")    # h4.T | h5.T
            c_st = {0: st.tile([B, HS], f32, tag="c0", name="c0"),
                    1: st.tile([B, HS], f32, tag="c1", name="c1"),
                    "A": st.tile([2 * B, HS], f32, tag="cA", name="cA"),
                    4: st.tile([B, HS], f32, tag="c4", name="c4"),
                    5: st.tile([B, HS], f32, tag="c5", name="c5")}
            x0b = st.tile([B, D_IN], f32, tag="x0b", name="x0b")
            x0T = st.tile([D_IN, B], f32r, tag="x0T", name="x0T")

            zf = wp.tile([128, 2 * B], f32, tag="zf", name="zf")
            nc.vector.memset(zf[:], 0.0)
            for c in range(NK):
                nc.scalar.copy(hTA[:, c, :], zf[:])
                nc.scalar.copy(hTL[:, c, :], zf[:])
            nc.vector.memset(c_st["A"][:], 0.0)
            nc.vector.memset(c_st[4][:], 0.0)
            nc.vector.memset(c_st[5][:], 0.0)

            r32 = lambda ap: ap.bitcast(f32r)

            def transpose_to(dst_dram_slices, src_sb, rows, cols):
                """src_sb [rows<=128, cols] -> transposed [cols, rows] written
                to dram col-splits."""
                done = 0
                while done < cols:
                    n = min(128, cols - done)
                    pt = pst.tile([128, 128], f32, tag="pt", name="pt")
                    nc.tensor.transpose(pt[0:n, 0:rows],
                                        src_sb[0:rows, done:done + n],
                                        ident[0:rows, 0:rows])
                    cp = wk.tile([128, 128], f32r, tag="tcp", name="tcp")
                    nc.scalar.copy(cp[0:n, 0:rows], pt[0:n, 0:rows])
                    for (dap, lo, hi) in dst_dram_slices:
                        nc.sync.dma_start(dap[done:done + n, :],
                                          cp[0:n, lo:hi])
                    done += n

            def allgather(n_rows):
                gin = dp.tile([n_rows, B], f32r, tag="agin", name="agin")
                gout = dp.tile([NC_ * n_rows, B], f32r, tag="agout", name="agout")
                return gin, gout

            def do_ag(gin, gout):
                nc.gpsimd.collective_compute(
                    "AllGather", OP.bypass, replica_groups=RG,
                    ins=[gin[:].opt()], outs=[gout[:].opt()])

            def dma_back(gout, dst, lo, hi):
                nc.sync.dma_start(
                    dst[:, :, lo:hi],
                    gout[:].rearrange("(c k) n -> k c n", k=128))

            # ---- prologue: states from host-computed means ----
            nc.sync.dma_start(
                c_st[0][:], bsec("cin", f32).rearrange("(p f) -> p f", p=B))
            nc.vector.tensor_copy(c_st[1][:], c_st[0][:])

            for (nm, dsts) in (("h0T", [(hT0, 0, B)]),
                               ("h1T", [(hT1, 0, B), (hT1, B, 2 * B)])):
                gin, gout = allgather(HS)
                nc.sync.dma_start(
                    gin[:], bsec(nm, f32r).rearrange("(p f) -> p f", p=HS))
                do_ag(gin, gout)
                for (dst, lo, hi) in dsts:
                    dma_back(gout, dst, lo, hi)

            # x0
            nc.sync.dma_start(
                x0b[:], bsec("p0", f32).rearrange("(p f) -> p f", p=B))
            ptp = pst.tile([128, 128], f32, tag="pt", name="pt")
            nc.tensor.transpose(ptp[0:D_IN, 0:B], x0b[0:B, 0:D_IN],
                                ident[0:B, 0:B])
            nc.scalar.copy(x0T[:], ptp[0:D_IN, 0:B])

            # ---- helpers for the recurrence ----
            def gate_mms(g0, g1, rows, wtag, x_chunks, h_chunks):
                first = True
                for (lhsT, wkey, c) in h_chunks + x_chunks:
                    if wkey == "0x":
                        r0 = w_sb["0x"][0:54, 0:288]
                        r1 = w_sb["0x"][0:54, 288:GS]
                    else:
                        r0 = w_sb[wkey][:, c, 0:288]
                        r1 = w_sb[wkey][:, c, 288:GS]
                    nc.tensor.matmul(g0[0:rows, :], r32(lhsT), r32(r0),
                                     start=first, stop=False)
                    nc.tensor.matmul(g1[0:rows, :], r32(lhsT), r32(r1),
                                     start=first, stop=False)
                    first = False
                nc.tensor.matmul(g0[0:rows, :], ones[0:1, 0:rows],
                                 b_sb[wtag][0:1, 0:288],
                                 start=False, stop=True)
                nc.tensor.matmul(g1[0:rows, :], ones[0:1, 0:rows],
                                 b_sb[wtag][0:1, 288:GS],
                                 start=False, stop=True)

            def elementwise(g0, g1, rows, c_tile, crange):
                """gates [i f | o g]; returns h_new sbuf tile [rows, HS]"""
                sif = wk.tile([128, 2 * HS], f32, tag="sif", name="sif")
                nc.scalar.activation(sif[0:rows, :], g0[0:rows, :], AF.Sigmoid)
                so = wk.tile([128, HS], f32, tag="so", name="so")
                nc.scalar.activation(so[0:rows, :], g1[0:rows, 0:HS], AF.Sigmoid)
                tg = wk.tile([128, HS], f32, tag="tg", name="tg")
                nc.scalar.activation(tg[0:rows, :], g1[0:rows, HS:2 * HS], AF.Tanh)
                t1 = wk.tile([128, HS], f32, tag="t1", name="t1")
                cs = c_tile[crange[0]:crange[1], :]
                nc.vector.tensor_tensor(t1[0:rows, :], sif[0:rows, HS:2 * HS],
                                        cs, op=OP.mult)
                t2 = wk.tile([128, HS], f32, tag="t2", name="t2")
                nc.vector.tensor_tensor(t2[0:rows, :], sif[0:rows, 0:HS],
                                        tg[0:rows, :], op=OP.mult)
                nc.vector.tensor_tensor(cs, t1[0:rows, :], t2[0:rows, :],
                                        op=OP.add)
                tc_ = wk.tile([128, HS], f32, tag="tc", name="tc")
                nc.scalar.activation(tc_[0:rows, :], cs, AF.Tanh)
                hn = hp.tile([128, HS], f32, tag="hnew", name="hnew")
                nc.vector.tensor_tensor(hn[0:rows, :], so[0:rows, :],
                                        tc_[0:rows, :], op=OP.mult)
                return hn

            # ---- recurrence ----
            for t in range(T_OUT):
                # L0
                g0 = psg.tile([128, 288], f32, tag="g0", name="g0")
                g1 = psg.tile([128, 288], f32, tag="g1", name="g1")
                gate_mms(g0, g1, B, "0",
                         x_chunks=[(x0T[0:54, 0:B], "0x", 0)],
                         h_chunks=[(hT0[:, c, :], "0h", c) for c in range(NK)])
                hn0 = elementwise(g0, g1, B, c_st[0], (0, B))
                gin0, gout0 = allgather(HS)
                transpose_to([(gin0[:], 0, B)], hn0, B, HS)
                do_ag(gin0, gout0)
                dma_back(gout0, hT0, 0, B)

                # L1 (x = new h0)
                g0 = psg.tile([128, 288], f32, tag="g0", name="g0")
                g1 = psg.tile([128, 288], f32, tag="g1", name="g1")
                gate_mms(g0, g1, B, "1",
                         x_chunks=[(hT0[:, c, :], "1x", c) for c in range(NK)],
                         h_chunks=[(hT1[:, c, 0:B], "1h", c) for c in range(NK)])
                hn1 = elementwise(g0, g1, B, c_st[1], (0, B))
                gin1, gout1 = allgather(HS)
                transpose_to([(gin1[:], 0, B)], hn1, B, HS)
                do_ag(gin1, gout1)
                dma_back(gout1, hT1, 0, B)
                dma_back(gout1, hT1, B, 2 * B)

                # A-pair: layers 2,3 stacked (x = new h1 for BOTH)
                g0 = psg.tile([128, 288], f32, tag="g0", name="g0")
                g1 = psg.tile([128, 288], f32, tag="g1", name="g1")
                gate_mms(g0, g1, 128, "A",
                         x_chunks=[(hT1[:, c, :], "Ax", c) for c in range(NK)],
                         h_chunks=[(hTA[:, c, :], "Ah", c) for c in range(NK)])
                hnA = elementwise(g0, g1, 128, c_st["A"], (0, 128))
                gin2, gout2 = allgather(HS)
                gin3, gout3 = allgather(HS)
                transpose_to([(gin2[:], 0, B), (gin3[:], B, 2 * B)],
                             hnA, 128, HS)
                do_ag(gin2, gout2)
                do_ag(gin3, gout3)
                dma_back(gout2, hTA, 0, B)
                dma_back(gout3, hTA, B, 2 * B)

                # L4 (x = new h3)
                g0 = psg.tile([128, 288], f32, tag="g0", name="g0")
                g1 = psg.tile([128, 288], f32, tag="g1", name="g1")
                gate_mms(g0, g1, B, "L",
                         x_chunks=[(hTA[:, c, B:2 * B], "Lx", c) for c in range(NK)],
                         h_chunks=[(hTL[:, c, 0:B], "Lh", c) for c in range(NK)])
                hn4 = elementwise(g0, g1, B, c_st[4], (0, B))
                gin4, gout4 = allgather(HS)
                transpose_to([(gin4[:], 0, B)], hn4, B, HS)
                do_ag(gin4, gout4)
                dma_back(gout4, hTL, 0, B)

                # L5 (x = new h4)
                g0 = psg.tile([128, 288], f32, tag="g0", name="g0")
                g1 = psg.tile([128, 288], f32, tag="g1", name="g1")
                gate_mms(g0, g1, B, "L",
                         x_chunks=[(hTL[:, c, 0:B], "Lx", c) for c in range(NK)],
                         h_chunks=[(hTL[:, c, B:2 * B], "Lh", c) for c in range(NK)])
                hn5 = elementwise(g0, g1, B, c_st[5], (0, B))
                gin5, gout5 = allgather(HS)
                transpose_to([(gin5[:], 0, B)], hn5, B, HS)
                do_ag(gin5, gout5)
                dma_back(gout5, hTL, B, 2 * B)

                # heads (replicated on every core)
                ph = psh.tile([B, D_IN], f32, tag="ph", name="ph")
                heads = [(hTA, 0, B, 0, 12),
                         (hTA, B, 2 * B, 12, 24),
                         (hT1, 0, B, 24, 36),
                         (hTL, 0, B, 36, 45),
                         (hTL, B, 2 * B, 45, 54)]
                for src, lo, hi, olo, ohi in heads:
                    for c in range(NK):
                        nc.tensor.matmul(ph[:, olo:ohi],
                                         src[:, c, lo:hi].bitcast(f32),
                                         wh_all[:, c, olo:ohi],
                                         start=(c == 0), stop=False)
                    nc.tensor.matmul(ph[:, olo:ohi], ones[0:1, 0:B],
                                     hb1[0:1, olo:ohi],
                                     start=False, stop=True)
                pre = wk.tile([B, D_IN], f32, tag="pre", name="pre")
                nc.vector.tensor_tensor(pre[:], ph[:], x0b[:], op=OP.add)
                pre_bf = wk.tile([B, D_IN], bf16, tag="prebf", name="prebf")
                nc.vector.tensor_copy(pre_bf[:], pre[:])
                nc.sync.dma_start(out_d[:, t, :], pre_bf[:])
                if t < T_OUT - 1:
                    nc.vector.tensor_copy(x0b[:], pre[:])
                    ptq = pst.tile([128, 128], f32, tag="pt", name="pt")
                    nc.tensor.transpose(ptq[0:D_IN, 0:B], pre[0:B, 0:D_IN],
                                        ident[0:B, 0:B])
                    nc.scalar.copy(x0T[:], ptq[0:D_IN, 0:B])

    nc.compile()
    return nc


def _quant(w):
    s = float(np.abs(w).max())
    if s == 0.0:
        s = 1.0
    q = np.clip(np.rint(w * (127.0 / s)), -127, 127).astype(np.int8)
    return q, np.float32(s / 127.0)


NF = NK * GS
NH = NF // 2
NL = NF // 4


def _prep_inputs(inputs):
    scales = np.zeros(24, np.float32)

    # seven big weight tensors -> int6 bit-planes per core
    wq6h = np.empty((NC_, 7, 128, NH), np.uint8)
    wq6l = np.empty((NC_, 7, 128, NL), np.uint8)
    for i, (tag, key) in enumerate(TAGS):
        W = inputs[key]
        s = float(np.abs(W).max())
        if s == 0.0:
            s = 1.0
        s6 = s / 31.0
        scales[3 * i] = 4.0 * s6
        scales[3 * i + 1] = -32.0 * s6
        scales[3 * i + 2] = s6
        v = (np.clip(np.rint(W * (1.0 / s6)), -31, 31) + 32.0).astype(np.uint8)
        # [4g, NC_, HS, NK, 128] -> per-core flat [NC_, 128(c), NK*GS]
        v5 = v.reshape(4, NC_, HS, NK, 128)[PERM]
        flat = v5.transpose(1, 4, 3, 0, 2).reshape(NC_, 128, NF)
        hpl = flat >> 2
        lpl = flat & 3
        wq6h[:, i] = hpl[:, :, 0:NH] | (hpl[:, :, NH:] << 4)
        wq6l[:, i] = (lpl[:, :, 0:NL] | (lpl[:, :, NL:2 * NL] << 2)
                      | (lpl[:, :, 2 * NL:3 * NL] << 4)
                      | (lpl[:, :, 3 * NL:] << 6))

    q, s = _quant(inputs["Wih0"])
    scales[21] = s
    w0x8 = q.reshape(4, NC_, HS, 54)[PERM].transpose(1, 3, 0, 2) \
            .reshape(NC_, 54, GS)

    whcat = np.concatenate([inputs["W_leg1"], inputs["W_leg2"],
                            inputs["W_spine"], inputs["W_arm1"],
                            inputs["W_arm2"]], axis=1).astype(np.float32)
    qh, sh = _quant(whcat)
    scales[22] = sh
    wh8 = np.ascontiguousarray(qh.reshape(NK, 128, 54))
    wscale = np.broadcast_to(scales, (128, 24)).copy()

    # biases per tag, per-core gate-col order
    b4 = np.stack([(inputs["bih" + t] + inputs["bhh" + t]).astype(np.float32)
                   for t in "01AL"])                       # [4tag, 4608]
    b4 = b4.reshape(4, 4, NC_, HS)[:, PERM]                # [tag, g', core, HS]
    b_all = np.ascontiguousarray(b4.transpose(2, 0, 1, 3).reshape(NC_, 4, GS))

    hbias = np.concatenate([inputs["b_leg1"], inputs["b_leg2"],
                            inputs["b_spine"], inputs["b_arm1"],
                            inputs["b_arm2"]]).astype(np.float32)[None, :]

    # host-side encoder means
    hs_sum = inputs["hidden_states"].sum(axis=1, dtype=np.float64)
    cin = (inputs["cell_states"].mean(axis=1, dtype=np.float64)).astype(np.float32)
    h0m = (hs_sum / T_ENC).astype(np.float32)
    h1m = ((hs_sum + inputs["global_t_state"]) / (T_ENC + 1)).astype(np.float32)
    h0T = np.ascontiguousarray(h0m.T)          # [H, B]
    h1T = np.ascontiguousarray(h1m.T)

    p0 = np.ascontiguousarray(inputs["p"][:, 0, :]).astype(np.float32)

    in_maps = []
    for j in range(NC_):
        sl = slice(j * HS, (j + 1) * HS)
        sec = {
            "wscale": wscale,
            "bias": b_all[j],
            "hb1": hbias,
            "h0T": h0T[sl],
            "h1T": h1T[sl],
            "cin": cin[:, sl],
            "p0": p0,
            "wh8": wh8,
            "w0x8": w0x8[j],
            "wq6h": wq6h[j],
            "wq6l": wq6l[j],
        }
        bl = np.empty(BLOB_BYTES, np.uint8)
        for name, arr in sec.items():
            off, nb = BLOB_OFF[name]
            flat = np.ascontiguousarray(arr).view(np.uint8).reshape(-1)
            assert flat.nbytes == nb, (name, flat.nbytes, nb)
            bl[off:off + nb] = flat
        in_maps.append({"blob": bl})
    return in_maps


def kernel(**inputs):
    global _compiled
    import concourse.bass_utils as bass_utils
    if _compiled is None:
        _compiled = _build()
    in_maps = _prep_inputs(inputs)
    res = bass_utils.run_bass_kernel_spmd(
        _compiled, in_maps, core_ids=list(range(NC_)))
    return np.asarray(res.results[0]["out"]).astype(np.float32)
